# revision 37
# baseline (speedup 1.0000x reference)
"""GRU decoder Bass kernel for Trainium2, data-parallel over batch on 8 cores.

Math refactoring (exactly equivalent to the reference up to fp assoc.):
  context = hidden[0] is constant across steps, and x_{t} = fc_out_{t-1} is
  linear in [h_t, context].  Folding fc into the input projection:
    gi_t = h_t @ M1.T + CONST          (M1 = W_ih @ fc_W[:, :H], t >= 1)
    gh_t = h_t @ W_hh.T + b_hh
  r/z gates add gi+gh, so P_r = M1_r + W_hh_r, P_z = M1_z + W_hh_z fuse into
  one [4096, 1024] weight:  G_t = h_t @ [P_r | P_z | M1_n | W_hh_n].T + C
  (33% fewer FLOPs than gi+gh).  fc_out_t = h_{t+1} @ F1.T + CF with
  F1 = fc_W[:, :H].  Step-0 pre-gates are computed on the host.

Schedule (~9.8us/step, PE ~96% busy, vs ~18.9us for the naive ordering):
  * G matmuls are emitted bank-major with all hta-chunk contraction pairs
    first: [r|z|h_n hta pairs, r|z|h_n htb pairs, i_n half A, i_n half B].
    Each bank's PSUM completes early enough for its gate ops to overlap
    the rest of the G burst, and the htb pairs start ~2.6us in, giving the
    previous step's half-B transpose+copy time to land without stalling.
  * h update uses h = (1-z)*n + z*h with 1-z = sigmoid(-zpre) on ACT.
    The elementwise tail runs in two free-dim halves, whole tail on DVE
    (concurrent DVE+GpSimd halves contend for SBUF ports at ~2x duration;
    GpSimd gets only the early z*h products), with explicit order-only
    dep edges pinning the DVE FIFO order against the list scheduler.
  * h is kept in fp16 only; transposes are 4x fp16 [128,128] PE transposes
    (each yields two hT chunks).  Half B's transposes + htb copy are
    deferred into the NEXT step's G stream (mid-G callback) so they can't
    block the step boundary in the in-order PE queue.
  * fc(t-1) (split 6/2 around T_A), the i_n/h_n PSUM constant preloads,
    and the transposes pack the PE gap while the gate tail runs — PE never
    idles >1.5us, so the HAM clock stays at 2.4GHz.  The r/z bias
    constants are fp32 adds on DVE instead of PSUM preloads (-0.4us PE).

Layout: per core B=64 batch rows, "split layout": every [64, 1024]
hidden-sized tensor is stored [128, 512] with dims 0-511 on partitions
0-63 (as batch rows) and dims 512-1023 on partitions 64-127.
Matmuls run in fp16 (weights + hT), fp32 accumulation in PSUM.  Per-batch
constants enter PSUM via an identity-weight matmul with fp16 hi+lo rows.
"""
import os
import numpy as np

H = 1024
OUT = 768
BATCH = 512
NCORES = 8
B = BATCH // NCORES  # 64

_BUILD_CACHE = {}

# hT chunk k (h dims 128k..128k+127, fp16, [128 part, 64 batch]) lives in
# hta (chunks 0,4,1,5) or htb (chunks 2,6,3,7) at these column offsets --
# the layout the [128,128] transposes + plain copies naturally produce.
_HT_SLOT = {0: ("a", 0), 4: ("a", 64), 1: ("a", 128), 5: ("a", 192),
            2: ("b", 0), 6: ("b", 64), 3: ("b", 128), 7: ("b", 192)}
# contraction order: half-A chunks first so G(t+1) can begin before the
# half-B transpose copies land.
_K_ORDER = [0, 1, 4, 5, 2, 3, 6, 7]


def _build(T: int):
    from contextlib import ExitStack
    from concourse import tile, mybir, bacc
    from concourse.tile import add_dep_helper

    F16 = mybir.dt.float16
    F32 = mybir.dt.float32
    Sig = mybir.ActivationFunctionType.Sigmoid
    Tanh = mybir.ActivationFunctionType.Tanh
    Copy = mybir.ActivationFunctionType.Copy
    MUL = mybir.AluOpType.mult
    ADD = mybir.AluOpType.add

    nc = bacc.Bacc("TRN2", target_bir_lowering=False, debug=False,
                   num_devices=NCORES)

    dram = {}
    def din(name, shape, dt):
        dram[name] = nc.dram_tensor(name, list(shape), dt, kind="ExternalInput").ap()
        return dram[name]

    w4_d = din("W4", [128, 8 * 4096], F16)
    f1_d = din("F1", [128, 8 * 768], F16)
    ci_d = din("CINIT", [128, 4 * 512], F16)
    crz_d = din("CRZ", [128, 1024], F32)
    cfs_d = din("CFS", [128, 384], F32)
    id2_d = din("IDENT2", [128, 64], F16)
    idt_d = din("IDENTT", [128, 128], F16)
    h0_d = din("H016", [128, 512], F16)
    g0_d = din("G0", [128, 4 * 512], F32)
    out_d = nc.dram_tensor("OUT", [T * 128, 384], F32, kind="ExternalOutput").ap()

    with tile.TileContext(nc) as tc:
        with ExitStack() as ctx:
            wpool = ctx.enter_context(tc.tile_pool(name="weights", bufs=1))
            state = ctx.enter_context(tc.tile_pool(name="state", bufs=1))
            tmp = ctx.enter_context(tc.tile_pool(name="tmp", bufs=2))
            stp = ctx.enter_context(tc.tile_pool(name="stp", bufs=3))
            gps = ctx.enter_context(tc.tile_pool(name="gpsum", bufs=1, space="PSUM"))
            tps = ctx.enter_context(tc.tile_pool(name="tpsum", bufs=1, space="PSUM"))
            fps = ctx.enter_context(tc.tile_pool(name="fpsum", bufs=1, space="PSUM"))

            w4 = wpool.tile([128, 8 * 4096], F16, name="w4")
            f1 = wpool.tile([128, 8 * 768], F16, name="f1")
            ci = wpool.tile([128, 4 * 512], F16, name="ci")
            crz = wpool.tile([128, 1024], F32, name="crz")
            cfs = wpool.tile([128, 384], F32, name="cfs")
            id2 = wpool.tile([128, 64], F16, name="id2")
            idt = wpool.tile([128, 128], F16, name="idt")
            g0 = wpool.tile([128, 4 * 512], F32, name="g0")
            h16 = state.tile([128, 512], F16, name="h16")
            hta = state.tile([128, 256], F16, name="hta")
            htb = state.tile([128, 256], F16, name="htb")

            for t_sb, t_d in ((w4, w4_d), (f1, f1_d), (ci, ci_d), (crz, crz_d),
                              (cfs, cfs_d), (id2, id2_d), (idt, idt_d),
                              (h16, h0_d), (g0, g0_d)):
                nc.sync.dma_start(t_sb[:], t_d[:])

            # PSUM banks: pre_r, pre_z, h_n full-width; i_n split in two
            # independent banks (half A = free cols 0:256, B = 256:512) so
            # the tail can read half A while PE still accumulates half B.
            gb0 = gps.tile([128, 512], F32, name="gb0", tag="gb0")
            gb1 = gps.tile([128, 512], F32, name="gb1", tag="gb1")
            gb3 = gps.tile([128, 512], F32, name="gb3", tag="gb3")
            gb2 = [gps.tile([128, 256], F32, name=f"gb2{h}", tag=f"gb2{h}",
                            padded_shape=[128, 512]) for h in range(2)]
            tpp = [tps.tile([128, 256], F16, name=f"tp{h}", tag=f"tp{h}",
                            padded_shape=[128, 1024]) for h in range(2)]
            fcp = fps.tile([128, 384], F32, name="fcp", tag="fcp",
                           padded_shape=[128, 512])

            def lhsT(k):
                ab, off = _HT_SLOT[k]
                t = hta if ab == "a" else htb
                return t[:, off:off + 64]

            BANKS013 = ((0, gb0), (1, gb1), (3, gb3))

            def emit_G(_, mid_cb=None):
                # Emission order: all hta-chunk pairs of the three N=512
                # banks first (so the previous step's htb transpose+copy has
                # ~2.6us of slack before the first htb-chunk matmul), then
                # their htb pairs, then the i_n halves.  r/z banks have no
                # PSUM preload: their first matmul starts the accumulation
                # group (bias constants are added on DVE instead).
                B512 = ((0, gb0, True), (1, gb1, True), (3, gb3, False))
                for j, gb, st0 in B512:
                    for ki, k in enumerate(_K_ORDER[:4]):
                        if j == 1 and ki == 3 and mid_cb is not None:
                            mid_cb()  # previous step's T_B, ~1.5us into G
                        lt = lhsT(k)
                        for g in range(2):
                            c0 = k * 4096 + (2 * j + g) * 512
                            nc.tensor.matmul(
                                gb[64 * g:64 * (g + 1), :], lt,
                                w4[:, c0:c0 + 512],
                                start=(st0 and ki == 0), stop=False)
                for j, gb, st0 in B512:
                    for ki, k in enumerate(_K_ORDER[4:]):
                        lt = lhsT(k)
                        for g in range(2):
                            c0 = k * 4096 + (2 * j + g) * 512
                            nc.tensor.matmul(
                                gb[64 * g:64 * (g + 1), :], lt,
                                w4[:, c0:c0 + 512],
                                start=False, stop=(ki == 3))
                for hf in range(2):
                    for ki, k in enumerate(_K_ORDER):
                        lt = lhsT(k)
                        for g in range(2):
                            c0 = k * 4096 + (4 + g) * 512 + hf * 256
                            nc.tensor.matmul(
                                gb2[hf][64 * g:64 * (g + 1), :], lt,
                                w4[:, c0:c0 + 256],
                                start=False, stop=(ki == 7))

            # CINIT layout: [p, (cc-4)*512 + c] for cc in 4..7
            # (i_n g0, i_n g1, h_n g0, h_n g1), hi rows p<64 / lo p>=64.
            def emit_init_hn(_):
                for g in range(2):
                    c0 = (2 + g) * 512
                    nc.tensor.matmul(gb3[64 * g:64 * (g + 1), :], id2[:, :],
                                     ci[:, c0:c0 + 512],
                                     start=True, stop=False)

            def emit_init2(hf):
                for g in range(2):
                    c0 = g * 512 + hf * 256
                    nc.tensor.matmul(gb2[hf][64 * g:64 * (g + 1), :], id2[:, :],
                                     ci[:, c0:c0 + 256],
                                     start=True, stop=False)

            def emit_fc_mms(t, half):
                for ki in (range(6) if half == 0 else range(6, 8)):
                    k = _K_ORDER[ki]
                    lt = lhsT(k)
                    for g in range(2):
                        c0 = k * 768 + g * 384
                        nc.tensor.matmul(
                            fcp[64 * g:64 * (g + 1), :], lt,
                            f1[:, c0:c0 + 384],
                            start=(ki == 0), stop=(ki == 7))

            def emit_st_dma(t, after=None):
                st = stp.tile([128, 384], F32, name=f"st{t}", tag="st")
                i_st = nc.vector.tensor_tensor(st[:], fcp[:], cfs[:, :], ADD)
                if after is not None:
                    # keep the output staging from sneaking ahead of the
                    # G-gating hta copy on the DVE FIFO
                    add_dep_helper(i_st.ins, after.ins, sync=False,
                                   reason="dve order: copyA before st")
                nc.sync.dma_start(out_d[t * 128:(t + 1) * 128, :], st[:])

            def emit_TB(t):
                for c in (2, 3):
                    nc.tensor.transpose(
                        tpp[1][:, (c % 2) * 128:(c % 2) * 128 + 128],
                        h16[:, c * 128:(c + 1) * 128], idt[:, :])
                return nc.vector.tensor_copy(htb[:, :], tpp[1][:, :])

            pending_tb = None
            last_cpb = None
            for t in range(T):
                if t == 0:
                    pr, pz = g0[:, 0:512], g0[:, 512:1024]
                    pin = (g0[:, 1024:1280], g0[:, 1280:1536])
                    phn = g0[:, 1536:2048]
                else:
                    # mid-G: previous step's T_B + htb copy (their results
                    # are only needed ~2.6us into this G), then the i_n-B
                    # constant preload consumed by this G's trailing MMs.
                    def mid(tb=pending_tb):
                        nonlocal last_cpb
                        last_cpb = tb()
                        emit_init2(1)
                    emit_G(t, mid_cb=mid)
                    pending_tb = None
                    pr, pz = gb0[:, :], gb1[:, :]
                    pin = (gb2[0][:, :], gb2[1][:, :])
                    phn = gb3[:, :]

                r = tmp.tile([128, 512], F32, name=f"r{t}", tag="r")
                z = tmp.tile([128, 512], F32, name=f"z{t}", tag="z")
                omz = tmp.tile([128, 512], F32, name=f"omz{t}", tag="omz")
                zh = tmp.tile([128, 512], F32, name=f"zh{t}", tag="zh")
                t1 = tmp.tile([128, 512], F32, name=f"t1{t}", tag="t1")

                if t == 0:
                    nc.scalar.activation(r[:], pr, Sig)
                    nc.scalar.activation(z[:], pz, Sig)
                    zsrc = pz
                else:
                    rp = tmp.tile([128, 512], F32, name=f"rp{t}", tag="rp")
                    zp = tmp.tile([128, 512], F32, name=f"zp{t}", tag="zp")
                    nc.vector.tensor_tensor(rp[:], pr, crz[:, 0:512], ADD)
                    nc.scalar.activation(r[:], rp[:], Sig)
                    nc.vector.tensor_tensor(zp[:], pz, crz[:, 512:1024], ADD)
                    nc.scalar.activation(z[:], zp[:], Sig)
                    zsrc = zp[:]
                # 1-z = sigmoid(-zpre) on ACT — keeps GpSimd down to the zh
                # halves, which finish before the DVE gate chain needs SBUF
                # ports (concurrent DVE+GpSimd ops run ~2x slower).
                nc.scalar.activation(omz[:], zsrc, Sig, scale=-1.0)
                # zh in halves so zh_A can't straggle past h16_A's need time
                nc.gpsimd.tensor_tensor(zh[:, 0:256], z[:, 0:256],
                                        h16[:, 0:256], MUL)
                nc.gpsimd.tensor_tensor(zh[:, 256:512], z[:, 256:512],
                                        h16[:, 256:512], MUL)
                nc.vector.tensor_tensor(t1[:], r[:], phn, MUL)

                if t + 1 < T:
                    emit_init_hn(t + 1)

                t2a = tmp.tile([128, 256], F32, name=f"t2a{t}", tag="t2a")
                t2b = tmp.tile([128, 256], F32, name=f"t2b{t}", tag="t2b")
                na = tmp.tile([128, 256], F32, name=f"na{t}", tag="na")
                nb = tmp.tile([128, 256], F32, name=f"nb{t}", tag="nb")
                aa = tmp.tile([128, 256], F32, name=f"aa{t}", tag="aa")
                ab = tmp.tile([128, 256], F32, name=f"ab{t}", tag="ab")
                slA, slB = slice(0, 256), slice(256, 512)
                # Whole tail on DVE (concurrent DVE+GpSimd halves contend
                # for SBUF ports, ~2x op duration).  A chain first — it
                # gates the next step's G; the B chain trails, using the
                # slack created by deferring T_B into the next G stream.
                nc.vector.tensor_tensor(t2a[:], t1[:, slA], pin[0], ADD)
                nc.scalar.activation(na[:], t2a[:], Tanh)
                nc.vector.tensor_tensor(aa[:], omz[:, slA], na[:], MUL)
                i_ha = nc.vector.tensor_tensor(h16[:, slA], aa[:], zh[:, slA],
                                               ADD)
                i_t2b = nc.vector.tensor_tensor(t2b[:], t1[:, slB], pin[1], ADD)
                add_dep_helper(i_t2b.ins, i_ha.ins, sync=False,
                               reason="dve order: A chain before t2b")
                nc.scalar.activation(nb[:], t2b[:], Tanh)
                # gb2[0] preload for t+1 — emitted after t2a/t2b's reads so
                # the program-order dataflow stays correct; PE-queue position
                # is still right after init_hn (no PE op in between).
                if t + 1 < T:
                    emit_init2(0)
                if t >= 1:
                    emit_fc_mms(t - 1, 0)
                for c in (0, 1):
                    nc.tensor.transpose(
                        tpp[0][:, c * 128:(c + 1) * 128],
                        h16[:, c * 128:(c + 1) * 128], idt[:, :])
                if t >= 1:
                    emit_fc_mms(t - 1, 1)
                i_cpa = nc.vector.tensor_copy(hta[:, :], tpp[0][:, :])
                add_dep_helper(i_cpa.ins, i_t2b.ins, sync=False,
                               reason="dve order: t2b before copyA")
                i_ab = nc.vector.tensor_tensor(ab[:], omz[:, slB], nb[:], MUL)
                add_dep_helper(i_ab.ins, i_cpa.ins, sync=False,
                               reason="dve order: copyA before ab")
                i_hb = nc.vector.tensor_tensor(h16[:, slB], ab[:], zh[:, slB],
                                               ADD)
                pending_tb = (lambda tt: (lambda: emit_TB(tt)))(t)

                if t >= 1:
                    emit_st_dma(t - 1, after=i_hb)

            if pending_tb is not None:
                pending_tb()
            emit_fc_mms(T - 1, 0)
            emit_fc_mms(T - 1, 1)
            emit_st_dma(T - 1)

    nc.compile()
    return nc


def _hi_lo(x):
    hi = x.astype(np.float16)
    lo = (x - hi.astype(np.float32)).astype(np.float16)
    return hi, lo


def _split_cols(x):
    """[B, 1024] -> [128, 512] split layout (dims 0-511 on parts 0-63)."""
    return np.concatenate([x[:, :512], x[:, 512:]], axis=0)


def kernel(src, hidden, W_ih, W_hh, b_ih, b_hh, fc_W, fc_b, output_len):
    from concourse import bass_utils

    T = int(output_len)
    src = np.asarray(src, np.float32)
    hidden = np.asarray(hidden, np.float32)
    W_ih = np.asarray(W_ih, np.float32)
    W_hh = np.asarray(W_hh, np.float32)
    b_ih = np.asarray(b_ih, np.float32)
    b_hh = np.asarray(b_hh, np.float32)
    fc_W = np.asarray(fc_W, np.float32)
    fc_b = np.asarray(fc_b, np.float32)

    ctx = hidden[0]          # [B, H]
    h0 = hidden[0]
    x0 = src[0]              # [B, OUT]

    # ---- host weight folding (fp32) ----
    M1 = W_ih @ fc_W[:, :H]          # [3H, H]
    M2 = W_ih @ fc_W[:, H:]          # [3H, H]
    P_r = M1[0:H] + W_hh[0:H]
    P_z = M1[H:2 * H] + W_hh[H:2 * H]
    Wbig4 = np.concatenate([P_r, P_z, M1[2 * H:], W_hh[2 * H:]], axis=0)  # [4096, H]
    F1 = fc_W[:, :H]                 # [OUT, H]

    CONST = ctx @ M2.T + (fc_b @ W_ih.T + b_ih)     # [B, 3H]
    c_r = CONST[:, 0:H] + b_hh[0:H]
    c_z = CONST[:, H:2 * H] + b_hh[H:2 * H]
    c_in = CONST[:, 2 * H:]
    c_hn = np.broadcast_to(b_hh[2 * H:], (BATCH, H)).astype(np.float32)
    CALL = np.stack([c_r, c_z, c_in, c_hn], axis=1)  # [B, 4, H]

    CF = ctx @ fc_W[:, H:].T + fc_b                  # [B, OUT]

    gi0 = x0 @ W_ih.T + b_ih
    gh0 = h0 @ W_hh.T + b_hh
    G0_parts = np.stack([gi0[:, :H] + gh0[:, :H],
                         gi0[:, H:2 * H] + gh0[:, H:2 * H],
                         gi0[:, 2 * H:],
                         gh0[:, 2 * H:]], axis=1)    # [B, 4, H]

    # ---- shared (replicated) tensors ----
    # W4 sbuf layout: [p, k*4096 + cc*512 + c] = Wbig4[1024*j + 512*g + c, 128k+p]
    W4T = Wbig4.T.reshape(8, 128, 8, 512)            # [k, p, cc, c]
    W4 = np.ascontiguousarray(W4T.transpose(1, 0, 2, 3)).reshape(128, 8 * 4096)
    W4 = W4.astype(np.float16)
    # F1 sbuf: [p, k*768 + g*384 + c] = F1[384g+c, 128k+p]
    F1T = F1.T.reshape(8, 128, 2, 384)               # [k, p, g, c]
    F1s = np.ascontiguousarray(F1T.transpose(1, 0, 2, 3)).reshape(128, 8 * 768)
    F1s = F1s.astype(np.float16)
    ID2 = np.concatenate([np.eye(64), np.eye(64)], axis=0).astype(np.float16)
    IDT = np.eye(128).astype(np.float16)

    key = T
    if key not in _BUILD_CACHE:
        _BUILD_CACHE[key] = _build(T)
    nc = _BUILD_CACHE[key]

    in_maps = []
    for c in range(NCORES):
        sl = slice(c * B, (c + 1) * B)
        # CINIT (i_n, h_n only): [p, ((j-2)*2+g)*512 + c]: p<64 hi, p>=64 lo
        call_c = CALL[sl][:, 2:4].reshape(B, 2, 2, 512)  # [b, j-2, g, c]
        hi, lo = _hi_lo(call_c)
        ci = np.concatenate([hi, lo], axis=0)        # [128, 2, 2, 512]
        ci = np.ascontiguousarray(ci).reshape(128, 4 * 512)

        # CRZ: fp32 [c_r | c_z] in split layout
        crz = np.concatenate([_split_cols(CALL[sl][:, 0]),
                              _split_cols(CALL[sl][:, 1])], axis=1)

        h0_c = h0[sl]
        H016 = _split_cols(h0_c).astype(np.float16)  # [128, 512]

        g0_c = G0_parts[sl]                          # [B, 4, H]
        G0s = np.concatenate([g0_c[:, :, :512], g0_c[:, :, 512:]],
                             axis=0)                 # [128, 4, 512]
        G0s = np.ascontiguousarray(G0s).reshape(128, 4 * 512)

        cf_c = CF[sl].reshape(B, 2, 384)             # [b, g, c]
        CFs = np.ascontiguousarray(cf_c.transpose(1, 0, 2)).reshape(128, 384)

        in_maps.append({
            "W4": W4, "F1": F1s,
            "CINIT": np.ascontiguousarray(ci).astype(np.float16),
            "CRZ": np.ascontiguousarray(crz).astype(np.float32),
            "CFS": CFs.astype(np.float32),
            "IDENT2": ID2, "IDENTT": IDT,
            "H016": np.ascontiguousarray(H016),
            "G0": G0s.astype(np.float32),
        })

    trace = bool(os.environ.get("GRU_TRACE"))
    res = bass_utils.run_bass_kernel_spmd(
        nc, in_maps, core_ids=list(range(NCORES)), trace=trace)
    if trace:
        kernel.last_exec_time_ns = res.exec_time_ns
        kernel.last_results = res

    outs = []
    for c in range(NCORES):
        o = res.results[c]["OUT"]                    # [T*128, 384]
        o = o.reshape(T, 2, B, 384).transpose(0, 2, 1, 3).reshape(T, B, OUT)
        outs.append(o)
    return np.concatenate(outs, axis=1)              # [T, BATCH, OUT]
